# revision 25
# baseline (speedup 1.0000x reference)
"""Fully-fused fp16 MoE expert FFN (E=8, C=2048, D=1024, F=4096), 8 TRN2 cores.

One expert per core; w1 AND w2 fully SBUF-resident in fp16. mm1 uses
one-level Strassen (7 half-size products instead of 8): per 512-token
chunk, 16 virtual f-blocks x 7 products x 4 k-steps of N=256 matmuls =
48.9us of PE vs 55.2us classic (N=256 matmuls stream at 109ns with
LDWEIGHTS fully hidden -- measured). B-combo strips are built on gpsimd
just-in-time per f-block, A-combos once per chunk on vector, and the
C-reconstruction adds run on vector ordered so PSUM banks free in
allocation order; gelu+bias stays fused in the scalar-engine eviction.
mm2 is classic (its A-side combos would not fit SBUF).

DMA: all inputs ride the pool SWDGE queue (async issue, ~340 GB/s, 4KB
packets) in consumption order; w1 streams in (left-half, right-half)
column pairs since Strassen consumes both f-halves together. sync
carries only b1 + output DMAs; scalar only the gelu ACTs. 34 dummy
N=256 matmuls on a memset tile bridge the HAM clock-gate window.
"""

import numpy as np

import concourse.bass as bass
import concourse.mybir as mybir
import concourse.tile as tile
from concourse import bacc
from concourse.bass_utils import run_bass_kernel_spmd

E, C, D, F = 8, 2048, 1024, 4096
P = 128
KD = D // P  # 8
MF = F // P  # 32
CN = C // 512  # 4 chunks of 512 tokens
CJ = 4  # 128-token subblocks per chunk
DN = D // 512  # 2
FJ = F // 512  # 8 column blocks of w1
JF = 16  # virtual f-blocks per chunk (Strassen half-F / 128)
WARMUP = 34

F32 = mybir.dt.float32
F16 = mybir.dt.float16
GELU = mybir.ActivationFunctionType.Gelu_apprx_tanh

_CACHE = {}


def _build():
    nc = bacc.Bacc("TRN2", target_bir_lowering=False, debug=False, num_devices=E)

    xh_d = nc.dram_tensor("xh", [CN, P, KD, 512], F16, kind="ExternalInput").ap()
    w1_d = nc.dram_tensor("w1h", [FJ, P, KD, 512], F16, kind="ExternalInput").ap()
    # w1's first column block split into 4x128 cols for startup granularity
    w1q_d = nc.dram_tensor("w1q", [4, P, KD, P], F16, kind="ExternalInput").ap()
    b1_d = nc.dram_tensor("b1t", [P, MF], F32, kind="ExternalInput").ap()
    w2_d = nc.dram_tensor("w2h", [DN, P, MF, 512], F16, kind="ExternalInput").ap()
    out_d = nc.dram_tensor("out", [C, D], F32, kind="ExternalOutput").ap()

    with tile.TileContext(nc) as tc:
        with (
            tc.tile_pool(name="w1f", bufs=1) as w1_pool,
            tc.tile_pool(name="w2f", bufs=1) as w2_pool,
            tc.tile_pool(name="b1", bufs=1) as b1_pool,
            tc.tile_pool(name="xt", bufs=2) as xt_pool,
            tc.tile_pool(name="ht", bufs=1) as ht_pool,
            tc.tile_pool(name="ev", bufs=2) as ev_pool,
            tc.tile_pool(name="evl", bufs=2) as evl_pool,
            tc.tile_pool(name="wrm", bufs=1) as wrm_pool,
            tc.tile_pool(name="rc", bufs=5) as rc_pool,
            tc.tile_pool(name="st", bufs=8) as st_pool,
            tc.tile_pool(name="rec", bufs=6) as rec_pool,
            tc.tile_pool(name="ps", bufs=8, space="PSUM") as ps_pool,
        ):
            # PE warmup through the HAM window
            wrm = wrm_pool.tile([P, 256], F16)
            nc.vector.memset(wrm[:], 0.0)
            for _ in range(WARMUP):
                wps = ps_pool.tile([P, 512], F32, tag="ps")
                nc.tensor.matmul(
                    wps[:, 0:256], wrm[:, 0:P], wrm[:], start=True, stop=True
                )

            b1t = b1_pool.tile([P, MF], F32)
            nc.sync.dma_start(b1t[:], b1_d[:])

            # pool SWDGE input stream, consumption order for Strassen mm1:
            # left-half cols of jj0 (per-128 via w1q), x0, then (right,left)
            # column-block pairs so each f-block's two halves coexist.
            w1f = w1_pool.tile([P, KD, F], F16)
            nc.gpsimd.dma_start(w1f[:, :, bass.ds(0, P)], w1q_d[0])

            xt0 = xt_pool.tile([P, KD, 512], F16, tag="xt")
            nc.gpsimd.dma_start(xt0[:, 0:4, :], xh_d[0, :, 0:4, :])
            nc.gpsimd.dma_start(xt0[:, 4:8, :], xh_d[0, :, 4:8, :])

            for jq in range(1, 4):
                nc.gpsimd.dma_start(w1f[:, :, bass.ds(jq * P, P)], w1q_d[jq])
            for jj in [4, 1, 5, 2, 6, 3, 7]:
                nc.gpsimd.dma_start(
                    w1f[:, :, bass.ds(jj * 512, 512)], w1_d[jj]
                )

            w2f = w2_pool.tile([P, MF, D], F16)

            def load_xt(cn):
                t = xt_pool.tile([P, KD, 512], F16, tag="xt")
                nc.gpsimd.dma_start(t[:], xh_d[cn])
                return t

            def acombos(xt):
                A11 = xt[:, 0:4, 0:256]
                A12 = xt[:, 4:8, 0:256]
                A21 = xt[:, 0:4, 256:512]
                A22 = xt[:, 4:8, 256:512]
                R = {}
                for name, op, a, b in (
                    ("r2", "add", A21, A22),
                    ("r1", "add", A11, A22),
                    ("r5", "add", A11, A12),
                    ("r6", "sub", A21, A11),
                    ("r7", "sub", A12, A22),
                ):
                    t = rc_pool.tile([P, 4, 256], F16, tag="rc")
                    (nc.vector.tensor_add if op == "add" else nc.vector.tensor_sub)(
                        t[:], a, b
                    )
                    R[name] = t
                return R

            R = acombos(xt0)
            xt = xt0
            xt1 = None
            for cn in range(CN):
                ht = ht_pool.tile([P, MF, 512], F16, tag="ht")
                A11 = xt[:, 0:4, 0:256]
                A22 = xt[:, 4:8, 256:512]
                for jf in range(JF):
                    jl, jr = jf * P, 2048 + jf * P
                    B11 = w1f[:, 0:4, bass.ds(jl, P)]
                    B12 = w1f[:, 0:4, bass.ds(jr, P)]
                    B21 = w1f[:, 4:8, bass.ds(jl, P)]
                    B22 = w1f[:, 4:8, bass.ds(jr, P)]
                    # B-combo strips (gpsimd), s4 first: it only needs the
                    # left half, which arrives earliest
                    strips = {}
                    for name, op, a, b in (
                        ("s4", "sub", B21, B11),
                        ("s1", "add", B11, B22),
                        ("s3", "sub", B12, B22),
                        ("s6", "add", B11, B12),
                        ("s7", "add", B21, B22),
                    ):
                        s = st_pool.tile([P, 4, P], F16, tag="st")
                        (
                            nc.gpsimd.tensor_add
                            if op == "add"
                            else nc.gpsimd.tensor_sub
                        )(s[:], a, b)
                        strips[name] = s
                    # late inputs ride the same queue between strip batches
                    if cn == 0 and jf == 1:
                        nc.gpsimd.dma_start(
                            w2f[:, :, bass.ds(0, 512)], w2_d[0]
                        )
                    if cn == 0 and jf == 3:
                        xt1 = load_xt(1)
                    if cn == 0 and jf == 5:
                        nc.gpsimd.dma_start(
                            w2f[:, :, bass.ds(512, 512)], w2_d[1]
                        )

                    def product(lhs_k, rhs_k):
                        ps = ps_pool.tile([P, 512], F32, tag="ps")
                        pv = ps[:, 0:256]
                        for k in range(4):
                            nc.tensor.matmul(
                                pv, lhs_k(k), rhs_k(k),
                                start=(k == 0), stop=(k == 3),
                            )
                        return ps

                    m2 = product(
                        lambda k: w1f[:, k, bass.ds(jl, P)],
                        lambda k: R["r2"][:, k, :],
                    )
                    m4 = product(
                        lambda k: strips["s4"][:, k, :],
                        lambda k: xt[:, 4 + k, 256:512],
                    )
                    m1 = product(
                        lambda k: strips["s1"][:, k, :],
                        lambda k: R["r1"][:, k, :],
                    )
                    m3 = product(
                        lambda k: strips["s3"][:, k, :],
                        lambda k: xt[:, k, 0:256],
                    )
                    m5 = product(
                        lambda k: w1f[:, 4 + k, bass.ds(jr, P)],
                        lambda k: R["r5"][:, k, :],
                    )
                    m6 = product(
                        lambda k: strips["s6"][:, k, :],
                        lambda k: R["r6"][:, k, :],
                    )
                    m7 = product(
                        lambda k: strips["s7"][:, k, :],
                        lambda k: R["r7"][:, k, :],
                    )

                    # reconstruction. DVE ops may read at most ONE psum
                    # operand, so m4/m1/m5 are evicted once via copies and
                    # the chains use those. Ops fire as each product stops,
                    # so bank recycling pipelines inside the jf.
                    def rt():
                        return rec_pool.tile(
                            [P, 256], F32, tag="rec", name="rec"
                        )

                    bl = b1t[:, jf : jf + 1]
                    br = b1t[:, JF + jf : JF + jf + 1]
                    e4 = rt()
                    nc.vector.tensor_copy(e4[:], m4[:, 0:256])
                    v = rt()  # C21 = M2 + M4
                    nc.vector.tensor_add(v[:], m2[:, 0:256], e4[:])
                    nc.scalar.activation(
                        ht[:, jf, 256:512], v[:], GELU, bias=bl
                    )
                    e1 = rt()
                    nc.vector.tensor_copy(e1[:], m1[:, 0:256])
                    u1 = rt()  # M1 - M2
                    nc.vector.tensor_sub(u1[:], e1[:], m2[:, 0:256])
                    t1 = rt()  # M1 + M4
                    nc.vector.tensor_add(t1[:], e1[:], e4[:])
                    e5 = rt()
                    nc.vector.tensor_copy(e5[:], m5[:, 0:256])
                    u = rt()  # C12 = M3 + M5
                    nc.vector.tensor_add(u[:], m3[:, 0:256], e5[:])
                    nc.scalar.activation(
                        ht[:, JF + jf, 0:256], u[:], GELU, bias=br
                    )
                    u2 = rt()
                    nc.vector.tensor_add(u2[:], u1[:], m3[:, 0:256])
                    t3 = rt()
                    nc.vector.tensor_sub(t3[:], t1[:], e5[:])
                    u3 = rt()  # C22 = M1 - M2 + M3 + M6
                    nc.vector.tensor_add(u3[:], u2[:], m6[:, 0:256])
                    nc.scalar.activation(
                        ht[:, JF + jf, 256:512], u3[:], GELU, bias=br
                    )
                    t4 = rt()  # C11 = M1 + M4 - M5 + M7
                    nc.vector.tensor_add(t4[:], t3[:], m7[:, 0:256])
                    nc.scalar.activation(
                        ht[:, jf, 0:256], t4[:], GELU, bias=bl
                    )

                # next chunk's activations + A-combos (vector runs them
                # during this chunk's mm2)
                if cn + 1 < CN:
                    xt = xt1 if cn == 0 else load_xt(cn + 1)
                    R = acombos(xt)

                for cj in range(CJ):
                    row = cn * 512 + cj * P
                    for dn in range(DN):
                        ps = ps_pool.tile([P, 512], F32, tag="ps")
                        for j in range(MF):
                            nc.tensor.matmul(
                                ps[:],
                                ht[:, j, bass.ds(cj * P, P)],
                                w2f[:, j, bass.ds(dn * 512, 512)],
                                start=(j == 0),
                                stop=(j == MF - 1),
                            )
                        last = cn == CN - 1 and cj == CJ - 1 and dn == DN - 1
                        if not last:
                            ev = ev_pool.tile([P, 512], F32, tag="ev")
                            nc.vector.tensor_copy(ev[:], ps[:])
                            nc.sync.dma_start(
                                out_d[row : row + P, dn * 512 : (dn + 1) * 512],
                                ev[:],
                            )
                        else:
                            for h in range(2):
                                evh = evl_pool.tile([P, 256], F32, tag="evl")
                                nc.vector.tensor_copy(
                                    evh[:], ps[:, bass.ds(h * 256, 256)]
                                )
                                col = dn * 512 + h * 256
                                nc.sync.dma_start(
                                    out_d[row : row + P, col : col + 256],
                                    evh[:],
                                )

    nc.compile()
    return nc


def _get_nc():
    if "nc" not in _CACHE:
        _CACHE["nc"] = _build()
    return _CACHE["nc"]


def _in_map(x_e, w1_e, b1_e, w2_e):
    xT = np.ascontiguousarray(x_e.T).astype(np.float16)  # [D, C]
    xh = np.ascontiguousarray(
        xT.reshape(KD, P, CN, 512).transpose(2, 1, 0, 3)
    )  # [CN, P, KD, 512]
    w1r = w1_e.astype(np.float16).reshape(KD, P, FJ, 512)
    w1h = np.ascontiguousarray(w1r.transpose(2, 1, 0, 3))  # [FJ, P, KD, 512]
    w1q = np.ascontiguousarray(
        w1h[0].reshape(P, KD, 4, P).transpose(2, 0, 1, 3)
    )  # [4, P, KD, 128]
    b1t = np.ascontiguousarray(b1_e.reshape(MF, P).T)
    w2r = w2_e.astype(np.float16).reshape(MF, P, DN, 512)
    w2h = np.ascontiguousarray(w2r.transpose(2, 1, 0, 3))  # [DN, P, MF, 512]
    return {"xh": xh, "w1h": w1h, "w1q": w1q, "b1t": b1t, "w2h": w2h}


def kernel(inputs, w1, b1, w2, b2, _trace=False):
    nc = _get_nc()
    x = np.asarray(inputs, dtype=np.float32).reshape(E, C, D)
    in_maps = [
        _in_map(
            x[e],
            np.asarray(w1[e], dtype=np.float32),
            np.asarray(b1[e], dtype=np.float32),
            np.asarray(w2[e], dtype=np.float32),
        )
        for e in range(E)
    ]
    res = run_bass_kernel_spmd(nc, in_maps, list(range(E)), trace=_trace)
    out = np.stack([res.results[e]["out"] for e in range(E)])[None]
    out = out + np.asarray(b2, dtype=np.float32)[None]
    if _trace:
        _CACHE["last_results"] = res
    return out.astype(np.float32)


# revision 26
# speedup vs baseline: 1.3694x; 1.3694x over previous
"""Fully-fused fp16 MoE expert FFN (E=8, C=2048, D=1024, F=4096), 8 TRN2 cores.

One expert per core; w1 AND w2 fully SBUF-resident in fp16. v3 changes vs
the serial-DMA baseline:
  - All input DMAs ride the gpsimd SWDGE queue as a handful of big merged
    instructions in exact consumption order (x0, w1 col-blocks, w2 halves,
    x1-3). SWDGE issue is async (~2us/instr) and the pool queue moves
    ~300+ GB/s with 4KB packets, so weights stream in well ahead of use --
    the baseline serialized 161 HWDGE instructions on sync at ~197 GB/s
    and starved mm2 of w2.
  - Host-side layouts match SBUF order so each piece is ONE instruction.
  - sync carries only b1 + output DMAs; scalar only the gelu evictions
    (an in-order engine with DMA backlog ahead of ACTs would stall PSUM
    recycling).
  - 20 dummy matmuls on a memset tile bridge the HAM activity window so
    real matmuls start at 2.4 GHz.
"""

import numpy as np

import concourse.bass as bass
import concourse.mybir as mybir
import concourse.tile as tile
from concourse import bacc
from concourse.bass_utils import run_bass_kernel_spmd

E, C, D, F = 8, 2048, 1024, 4096
P = 128
KD = D // P  # 8
MF = F // P  # 32
CN = C // 512  # 4 chunks of 512 tokens
CJ = 4  # 128-token subblocks per chunk
DN = D // 512  # 2
FJ = F // 512  # 8 column blocks of w1
WARMUP = 26

F32 = mybir.dt.float32
F16 = mybir.dt.float16
GELU = mybir.ActivationFunctionType.Gelu_apprx_tanh

_CACHE = {}


def _build():
    nc = bacc.Bacc("TRN2", target_bir_lowering=False, debug=False, num_devices=E)

    # Layouts are pre-transposed on host so every DMA below is a single
    # instruction whose src/dst iteration orders match.
    xh_d = nc.dram_tensor("xh", [CN, P, KD, 512], F16, kind="ExternalInput").ap()
    w1_d = nc.dram_tensor("w1h", [FJ, P, KD, 512], F16, kind="ExternalInput").ap()
    # duplicate of w1's first column block, split into 4x128 cols for startup
    w1q_d = nc.dram_tensor("w1q", [4, P, KD, P], F16, kind="ExternalInput").ap()
    b1_d = nc.dram_tensor("b1t", [P, MF], F32, kind="ExternalInput").ap()
    w2_d = nc.dram_tensor("w2h", [DN, P, MF, 512], F16, kind="ExternalInput").ap()
    out_d = nc.dram_tensor("out", [C, D], F32, kind="ExternalOutput").ap()

    with tile.TileContext(nc) as tc:
        with (
            tc.tile_pool(name="w1f", bufs=1) as w1_pool,
            tc.tile_pool(name="w2f", bufs=1) as w2_pool,
            tc.tile_pool(name="b1", bufs=1) as b1_pool,
            tc.tile_pool(name="xt", bufs=3) as xt_pool,
            tc.tile_pool(name="ht", bufs=1) as ht_pool,
            tc.tile_pool(name="ev", bufs=4) as ev_pool,
            tc.tile_pool(name="evl", bufs=2) as evl_pool,
            tc.tile_pool(name="wrm", bufs=1) as wrm_pool,
            tc.tile_pool(name="ps1", bufs=4, space="PSUM") as ps1_pool,
            tc.tile_pool(name="ps2", bufs=4, space="PSUM") as ps2_pool,
        ):
            # PE warmup: memset a dummy tile, then a stream of matmuls on it
            # so the HAM clock-gate opens before real data arrives.
            wrm = wrm_pool.tile([P, 512], F16)
            nc.vector.memset(wrm[:], 0.0)
            for _ in range(WARMUP):
                wps = ps2_pool.tile([P, 512], F32, tag="ps2")
                nc.tensor.matmul(wps[:], wrm[:, 0:P], wrm[:], start=True, stop=True)

            b1t = b1_pool.tile([P, MF], F32)
            nc.sync.dma_start(b1t[:], b1_d[:])

            # Input stream on the pool (SWDGE) queue, in consumption order.
            # j0's w1 column first (256KB), then x0, then j1-j3 columns, so
            # the first mm1 group starts the moment x0 lands.
            w1f = w1_pool.tile([P, KD, F], F16)
            nc.gpsimd.dma_start(w1f[:, :, bass.ds(0, P)], w1q_d[0])

            xt0 = xt_pool.tile([P, KD, 512], F16, tag="xt")
            nc.gpsimd.dma_start(xt0[:, 0:4, :], xh_d[0, :, 0:4, :])
            nc.gpsimd.dma_start(xt0[:, 4:8, :], xh_d[0, :, 4:8, :])

            for jq in range(1, 4):
                nc.gpsimd.dma_start(
                    w1f[:, :, bass.ds(jq * P, P)], w1q_d[jq]
                )
            for jj in range(1, FJ):
                nc.gpsimd.dma_start(
                    w1f[:, :, bass.ds(jj * 512, 512)], w1_d[jj]
                )

            def load_xt(cn):
                t = xt_pool.tile([P, KD, 512], F16, tag="xt")
                nc.gpsimd.dma_start(t[:], xh_d[cn])
                return t

            w2f = w2_pool.tile([P, MF, D], F16)
            nc.gpsimd.dma_start(w2f[:, :, bass.ds(0, 512)], w2_d[0])
            xt1 = load_xt(1)
            nc.gpsimd.dma_start(w2f[:, :, bass.ds(512, 512)], w2_d[1])

            xt = xt0
            for cn in range(CN):
                ht = ht_pool.tile([P, MF, 512], F16, tag="ht")
                for j in range(MF):
                    ps = ps1_pool.tile([P, 512], F32, tag="ps1")
                    for k in range(KD):
                        nc.tensor.matmul(
                            ps[:],
                            w1f[:, k, bass.ds(j * P, P)],
                            xt[:, k, :],
                            start=(k == 0),
                            stop=(k == KD - 1),
                        )
                    nc.scalar.activation(
                        ht[:, j, :], ps[:], GELU, bias=b1t[:, j : j + 1]
                    )
                # prefetch next chunk (x1 was already queued before w2dn1)
                if cn + 1 < CN:
                    xt = xt1 if cn == 0 else load_xt(cn + 1)
                for cj in range(CJ):
                    row = cn * 512 + cj * P
                    for dn in range(DN):
                        ps = ps2_pool.tile([P, 512], F32, tag="ps2")
                        for j in range(MF):
                            nc.tensor.matmul(
                                ps[:],
                                ht[:, j, bass.ds(cj * P, P)],
                                w2f[:, j, bass.ds(dn * 512, 512)],
                                start=(j == 0),
                                stop=(j == MF - 1),
                            )
                        last = cn == CN - 1 and cj == CJ - 1 and dn == DN - 1
                        if not last:
                            ev = ev_pool.tile([P, 512], F32, tag="ev")
                            nc.vector.tensor_copy(ev[:], ps[:])
                            nc.sync.dma_start(
                                out_d[row : row + P, dn * 512 : (dn + 1) * 512],
                                ev[:],
                            )
                        else:
                            # split the final eviction so its DMA starts
                            # ~1us sooner (shorter kernel tail)
                            for h in range(2):
                                evh = evl_pool.tile([P, 256], F32, tag="evl")
                                nc.vector.tensor_copy(
                                    evh[:], ps[:, bass.ds(h * 256, 256)]
                                )
                                col = dn * 512 + h * 256
                                nc.sync.dma_start(
                                    out_d[row : row + P, col : col + 256],
                                    evh[:],
                                )

    nc.compile()
    return nc


def _get_nc():
    if "nc" not in _CACHE:
        _CACHE["nc"] = _build()
    return _CACHE["nc"]


def _in_map(x_e, w1_e, b1_e, w2_e):
    xT = np.ascontiguousarray(x_e.T).astype(np.float16)  # [D, C]
    xh = np.ascontiguousarray(
        xT.reshape(KD, P, CN, 512).transpose(2, 1, 0, 3)
    )  # [CN, P, KD, 512]
    w1r = w1_e.astype(np.float16).reshape(KD, P, FJ, 512)
    w1h = np.ascontiguousarray(w1r.transpose(2, 1, 0, 3))  # [FJ, P, KD, 512]
    w1q = np.ascontiguousarray(
        w1h[0].reshape(P, KD, 4, P).transpose(2, 0, 1, 3)
    )  # [4, P, KD, 128]
    b1t = np.ascontiguousarray(b1_e.reshape(MF, P).T)
    w2r = w2_e.astype(np.float16).reshape(MF, P, DN, 512)
    w2h = np.ascontiguousarray(w2r.transpose(2, 1, 0, 3))  # [DN, P, MF, 512]
    return {"xh": xh, "w1h": w1h, "w1q": w1q, "b1t": b1t, "w2h": w2h}


def kernel(inputs, w1, b1, w2, b2, _trace=False):
    nc = _get_nc()
    x = np.asarray(inputs, dtype=np.float32).reshape(E, C, D)
    in_maps = [
        _in_map(
            x[e],
            np.asarray(w1[e], dtype=np.float32),
            np.asarray(b1[e], dtype=np.float32),
            np.asarray(w2[e], dtype=np.float32),
        )
        for e in range(E)
    ]
    res = run_bass_kernel_spmd(nc, in_maps, list(range(E)), trace=_trace)
    out = np.stack([res.results[e]["out"] for e in range(E)])[None]
    out = out + np.asarray(b2, dtype=np.float32)[None]
    if _trace:
        _CACHE["last_results"] = res
    return out.astype(np.float32)


# revision 34
# speedup vs baseline: 1.3696x; 1.0002x over previous
"""Fully-fused fp16 MoE expert FFN (E=8, C=2048, D=1024, F=4096), 8 TRN2 cores.

One expert per core; w1 AND w2 fully SBUF-resident in fp16. v3 changes vs
the serial-DMA baseline:
  - All input DMAs ride the gpsimd SWDGE queue as a handful of big merged
    instructions in exact consumption order (x0, w1 col-blocks, w2 halves,
    x1-3). SWDGE issue is async (~2us/instr) and the pool queue moves
    ~300+ GB/s with 4KB packets, so weights stream in well ahead of use --
    the baseline serialized 161 HWDGE instructions on sync at ~197 GB/s
    and starved mm2 of w2.
  - Host-side layouts match SBUF order so each piece is ONE instruction.
  - sync carries only b1 + output DMAs; scalar only the gelu evictions
    (an in-order engine with DMA backlog ahead of ACTs would stall PSUM
    recycling).
  - 20 dummy matmuls on a memset tile bridge the HAM activity window so
    real matmuls start at 2.4 GHz.
"""

import numpy as np

import concourse.bass as bass
import concourse.mybir as mybir
import concourse.tile as tile
from concourse import bacc
from concourse.bass_utils import run_bass_kernel_spmd

E, C, D, F = 8, 2048, 1024, 4096
P = 128
KD = D // P  # 8
MF = F // P  # 32
CN = C // 512  # 4 chunks of 512 tokens
CJ = 4  # 128-token subblocks per chunk
DN = D // 512  # 2
FJ = F // 512  # 8 column blocks of w1
WARMUP = 18

F32 = mybir.dt.float32
F16 = mybir.dt.float16
GELU = mybir.ActivationFunctionType.Gelu_apprx_tanh

_CACHE = {}


def _build():
    nc = bacc.Bacc("TRN2", target_bir_lowering=False, debug=False, num_devices=E)

    # Layouts are pre-transposed on host so every DMA below is a single
    # instruction whose src/dst iteration orders match.
    xh_d = nc.dram_tensor("xh", [CN, P, KD, 512], F16, kind="ExternalInput").ap()
    w1_d = nc.dram_tensor("w1h", [FJ, P, KD, 512], F16, kind="ExternalInput").ap()
    # duplicate of w1's first column block, split into 4x128 cols for startup
    w1q_d = nc.dram_tensor("w1q", [4, P, KD, P], F16, kind="ExternalInput").ap()
    # duplicate of x chunk 0, split into token halves for startup
    xq_d = nc.dram_tensor("xq", [2, P, KD, 256], F16, kind="ExternalInput").ap()
    b1_d = nc.dram_tensor("b1t", [P, MF], F32, kind="ExternalInput").ap()
    w2_d = nc.dram_tensor("w2h", [DN, P, MF, 512], F16, kind="ExternalInput").ap()
    out_d = nc.dram_tensor("out", [C, D], F32, kind="ExternalOutput").ap()

    with tile.TileContext(nc) as tc:
        with (
            tc.tile_pool(name="w1f", bufs=1) as w1_pool,
            tc.tile_pool(name="w2f", bufs=1) as w2_pool,
            tc.tile_pool(name="b1", bufs=1) as b1_pool,
            tc.tile_pool(name="xt", bufs=3) as xt_pool,
            tc.tile_pool(name="ht", bufs=1) as ht_pool,
            tc.tile_pool(name="ev", bufs=4) as ev_pool,
            tc.tile_pool(name="evl", bufs=2) as evl_pool,
            tc.tile_pool(name="wrm", bufs=1) as wrm_pool,
            tc.tile_pool(name="ps1", bufs=4, space="PSUM") as ps1_pool,
            tc.tile_pool(name="ps2", bufs=4, space="PSUM") as ps2_pool,
        ):
            # PE warmup: memset a dummy tile, then a stream of matmuls on it
            # so the HAM clock-gate opens before real data arrives.
            wrm = wrm_pool.tile([P, 512], F16)
            nc.vector.memset(wrm[:], 0.0)
            for _ in range(WARMUP):
                wps = ps2_pool.tile([P, 512], F32, tag="ps2")
                nc.tensor.matmul(wps[:], wrm[:, 0:P], wrm[:], start=True, stop=True)

            b1t = b1_pool.tile([P, MF], F32)
            nc.sync.dma_start(b1t[:], b1_d[:])

            # Input stream on the pool (SWDGE) queue, in consumption order.
            # j0's w1 column, x0's first token half, j1's column, x0's
            # second half, j2/j3 -- so token-halved j-groups start the
            # moment each 512KB piece lands.
            w1f = w1_pool.tile([P, KD, F], F16)
            nc.gpsimd.dma_start(w1f[:, :, bass.ds(0, P)], w1q_d[0])

            xt0 = xt_pool.tile([P, KD, 512], F16, tag="xt")
            nc.gpsimd.dma_start(xt0[:, :, 0:256], xq_d[0])
            nc.gpsimd.dma_start(w1f[:, :, bass.ds(P, P)], w1q_d[1])
            nc.gpsimd.dma_start(xt0[:, :, 256:512], xq_d[1])

            for jq in range(2, 4):
                nc.gpsimd.dma_start(
                    w1f[:, :, bass.ds(jq * P, P)], w1q_d[jq]
                )
            for jj in range(1, FJ):
                nc.gpsimd.dma_start(
                    w1f[:, :, bass.ds(jj * 512, 512)], w1_d[jj]
                )

            def load_xt(cn):
                t = xt_pool.tile([P, KD, 512], F16, tag="xt")
                nc.gpsimd.dma_start(t[:], xh_d[cn])
                return t

            w2f = w2_pool.tile([P, MF, D], F16)
            nc.gpsimd.dma_start(w2f[:, :, bass.ds(0, 512)], w2_d[0])
            xt1 = load_xt(1)
            nc.gpsimd.dma_start(w2f[:, :, bass.ds(512, 512)], w2_d[1])

            xt = xt0
            for cn in range(CN):
                ht = ht_pool.tile([P, MF, 512], F16, tag="ht")
                jstart = 0
                if cn == 0:
                    # first four j-groups token-halved: the 'a' halves need
                    # only x0's first 512KB, pulling the stream start in by
                    # ~2us. Both halves share one psum bank per j, so the
                    # eviction is still one full-width ACT.
                    jstart = 4
                    pss = []
                    for j in range(jstart):
                        ps = ps1_pool.tile([P, 512], F32, tag="ps1")
                        for k in range(KD):
                            nc.tensor.matmul(
                                ps[:, 0:256],
                                w1f[:, k, bass.ds(j * P, P)],
                                xt[:, k, 0:256],
                                start=(k == 0),
                                stop=(k == KD - 1),
                            )
                        pss.append(ps)
                    for j in range(jstart):
                        ps = pss[j]
                        for k in range(KD):
                            nc.tensor.matmul(
                                ps[:, 256:512],
                                w1f[:, k, bass.ds(j * P, P)],
                                xt[:, k, 256:512],
                                start=(k == 0),
                                stop=(k == KD - 1),
                            )
                        nc.scalar.activation(
                            ht[:, j, :], ps[:], GELU, bias=b1t[:, j : j + 1]
                        )
                for j in range(jstart, MF):
                    ps = ps1_pool.tile([P, 512], F32, tag="ps1")
                    for k in range(KD):
                        nc.tensor.matmul(
                            ps[:],
                            w1f[:, k, bass.ds(j * P, P)],
                            xt[:, k, :],
                            start=(k == 0),
                            stop=(k == KD - 1),
                        )
                    nc.scalar.activation(
                        ht[:, j, :], ps[:], GELU, bias=b1t[:, j : j + 1]
                    )
                # prefetch next chunk (x1 was already queued before w2dn1)
                if cn + 1 < CN:
                    xt = xt1 if cn == 0 else load_xt(cn + 1)
                for cj in range(CJ):
                    row = cn * 512 + cj * P
                    for dn in range(DN):
                        last = cn == CN - 1 and cj == CJ - 1 and dn == DN - 1
                        ps = ps2_pool.tile([P, 512], F32, tag="ps2")
                        if not last:
                            for j in range(MF):
                                nc.tensor.matmul(
                                    ps[:],
                                    ht[:, j, bass.ds(cj * P, P)],
                                    w2f[:, j, bass.ds(dn * 512, 512)],
                                    start=(j == 0),
                                    stop=(j == MF - 1),
                                )
                            ev = ev_pool.tile([P, 512], F32, tag="ev")
                            nc.vector.tensor_copy(ev[:], ps[:])
                            nc.sync.dma_start(
                                out_d[row : row + P, dn * 512 : (dn + 1) * 512],
                                ev[:],
                            )
                        else:
                            # final group as two d-halves in one bank so the
                            # first eviction+DMA overlaps the second half's
                            # matmuls (shorter kernel tail)
                            for h in range(2):
                                for j in range(MF):
                                    nc.tensor.matmul(
                                        ps[:, bass.ds(h * 256, 256)],
                                        ht[:, j, bass.ds(cj * P, P)],
                                        w2f[
                                            :, j,
                                            bass.ds(dn * 512 + h * 256, 256),
                                        ],
                                        start=(j == 0),
                                        stop=(j == MF - 1),
                                    )
                                evh = evl_pool.tile([P, 256], F32, tag="evl")
                                nc.vector.tensor_copy(
                                    evh[:], ps[:, bass.ds(h * 256, 256)]
                                )
                                col = dn * 512 + h * 256
                                nc.sync.dma_start(
                                    out_d[row : row + P, col : col + 256],
                                    evh[:],
                                )

    nc.compile()
    return nc


def _get_nc():
    if "nc" not in _CACHE:
        _CACHE["nc"] = _build()
    return _CACHE["nc"]


def _in_map(x_e, w1_e, b1_e, w2_e):
    xT = np.ascontiguousarray(x_e.T).astype(np.float16)  # [D, C]
    xh = np.ascontiguousarray(
        xT.reshape(KD, P, CN, 512).transpose(2, 1, 0, 3)
    )  # [CN, P, KD, 512]
    w1r = w1_e.astype(np.float16).reshape(KD, P, FJ, 512)
    w1h = np.ascontiguousarray(w1r.transpose(2, 1, 0, 3))  # [FJ, P, KD, 512]
    w1q = np.ascontiguousarray(
        w1h[0].reshape(P, KD, 4, P).transpose(2, 0, 1, 3)
    )  # [4, P, KD, 128]
    xq = np.ascontiguousarray(
        xh[0].reshape(P, KD, 2, 256).transpose(2, 0, 1, 3)
    )  # [2, P, KD, 256]
    b1t = np.ascontiguousarray(b1_e.reshape(MF, P).T)
    w2r = w2_e.astype(np.float16).reshape(MF, P, DN, 512)
    w2h = np.ascontiguousarray(w2r.transpose(2, 1, 0, 3))  # [DN, P, MF, 512]
    return {"xh": xh, "w1h": w1h, "w1q": w1q, "xq": xq, "b1t": b1t, "w2h": w2h}


def kernel(inputs, w1, b1, w2, b2, _trace=False):
    nc = _get_nc()
    x = np.asarray(inputs, dtype=np.float32).reshape(E, C, D)
    in_maps = [
        _in_map(
            x[e],
            np.asarray(w1[e], dtype=np.float32),
            np.asarray(b1[e], dtype=np.float32),
            np.asarray(w2[e], dtype=np.float32),
        )
        for e in range(E)
    ]
    res = run_bass_kernel_spmd(nc, in_maps, list(range(E)), trace=_trace)
    out = np.stack([res.results[e]["out"] for e in range(E)])[None]
    out = out + np.asarray(b2, dtype=np.float32)[None]
    if _trace:
        _CACHE["last_results"] = res
    return out.astype(np.float32)
